# revision 23
# baseline (speedup 1.0000x reference)
"""Sparse attention (B=8,H=8,LQ=LK=1024,D=64) on 8 TRN2 NeuronCores.

Strategy: shard the 64 (b,h) pairs across 8 cores (8 pairs/core) — fully
independent, no collectives. On each core, compute in the TRANSPOSED
domain: scores^T [k, q] tiles so that the P@V contraction (over k) needs
no on-chip transposes; Q^T / K^T / mask^T are produced host-side during
sharding (pure layout), outputs are un-transposed host-side after gather.

Per (b,h) pair on device:
  S^T[k,q]   = K^T_tile.T @ Q^T     (PE, f32r)
  P          = exp(S^T / 8)         (ScalarE, PSUM->SBUF, bf16 out)
  Pm         = P * mask^T           (VectorE, int32 operand, in-place)
  out'^T     = [V | 1].T @ Pm       (PE accumulate; row 64 = rowsums)
  recip      = 1/rowsums, redistributed [1,1024]->[128,8] via tiny MMs
  R          = broadcast(recip * qmask) across partitions via diag-matmul
  att^T      = Pm * R_bf16          (VectorE 2x mode, bf16 out) -> DMA
  out^T      = out'^T[0:64] * R[0:64] -> DMA

Emission is software-pipelined and interleaved at k-tile granularity:
pair p-1's normalize work is emitted tile-by-tile between pair p's
tiles so every engine's in-order stream always has ready work.
"""

import numpy as np

B, H, LQ, LK, D = 8, 8, 1024, 1024, 64
N_CORES = 8
PAIRS = (B * H) // N_CORES          # 8 pairs per core
KT = LK // 128                      # 8 k-tiles of 128
SCALE = 1.0 / 8.0                   # 1/sqrt(64)

_compiled = {}


def _build_bass():
    import concourse.tile as tile
    import concourse.bacc as bacc
    import concourse.mybir as mybir
    from concourse.masks import make_identity

    F32 = mybir.dt.float32
    F32R = mybir.dt.float32r
    BF16 = mybir.dt.bfloat16
    I32 = mybir.dt.int32
    AF = mybir.ActivationFunctionType

    nc = bacc.Bacc("TRN2", target_bir_lowering=False, debug=False,
                   num_devices=N_CORES)

    qT = nc.dram_tensor("qT", [PAIRS, D, LQ], F32R, kind="ExternalInput")
    kT = nc.dram_tensor("kT", [PAIRS, D, LK], F32R, kind="ExternalInput")
    v = nc.dram_tensor("v", [PAIRS, 128, KT, D + 1], F32, kind="ExternalInput")
    maskT = nc.dram_tensor("maskT", [PAIRS, 128, KT, LQ], I32, kind="ExternalInput")
    qmc = nc.dram_tensor("qmc", [PAIRS, 128, 512], F32, kind="ExternalInput")
    attT = nc.dram_tensor("attT", [PAIRS, LK, LQ], BF16, kind="ExternalOutput")
    outT = nc.dram_tensor("outT", [PAIRS, D, LQ], F32, kind="ExternalOutput")

    with tile.TileContext(nc) as tc:
        with (
            tc.tile_pool(name="constp", bufs=1) as constp,
            tc.tile_pool(name="qkp", bufs=2) as qkp,
            tc.tile_pool(name="vp", bufs=2) as vp,
            tc.tile_pool(name="maskp", bufs=2) as maskp,
            tc.tile_pool(name="pmp", bufs=2) as pmp,
            tc.tile_pool(name="attp", bufs=3) as attp,
            tc.tile_pool(name="smallp", bufs=2) as smallp,
            tc.tile_pool(name="ps_sc", bufs=4, space="PSUM") as ps_sc,
            tc.tile_pool(name="ps_rr", bufs=1, space="PSUM") as ps_rr,
            tc.tile_pool(name="ps_pv", bufs=1, space="PSUM") as ps_pv,
        ):
            # constants
            ones_f32 = constp.tile([128, 128], F32)
            nc.vector.memset(ones_f32[:], 1.0)
            ones128_b = constp.tile([128, 128], BF16)
            nc.vector.tensor_copy(ones128_b[:], ones_f32[:])

            # HAM warmup: ~5us of back-to-back matmuls so the PE clock
            # ungates to 2.4GHz before real work begins.
            wu_rhs = constp.tile([128, 512], BF16)
            nc.vector.memset(wu_rhs[:], 0.5)
            wu_ps = ps_sc.tile([128, 512], F32, tag="ps", name="wu_ps")
            for _ in range(12):
                nc.tensor.matmul(wu_ps[:], ones128_b[:], wu_rhs[:],
                                 start=True, stop=True)

            st = [dict() for _ in range(PAIRS)]   # per-pair live tiles

            def load(p):
                s = st[p]
                s["qt"] = qkp.tile([D, LQ], F32R, tag="qt", name=f"qt{p}")
                s["kt"] = qkp.tile([D, LK], F32R, tag="kt", name=f"kt{p}")
                s["vt"] = vp.tile([128, KT, D + 1], BF16, tag="vt", name=f"vt{p}")
                # bf16 tile filled by SWDGE cast-DMA from the int32 mask —
                # HBM reads unchanged, halves SBUF + enables DVE 2x mode
                s["mk"] = maskp.tile([128, KT, LQ], BF16, tag="mk", name=f"mk{p}")
                s["qmt"] = smallp.tile([128, 512], F32, tag="qmt", name=f"qmt{p}")
                nc.sync.dma_start(s["qt"][:], qT[p])
                nc.sync.dma_start(s["kt"][:], kT[p])
                nc.sync.dma_start(s["qmt"][:], qmc[p])
                nc.gpsimd.dma_start(s["vt"][:], v[p])   # SWDGE cast f32->bf16
                # mask per k-tile so the first tile's compute starts early
                for c in range(KT):
                    nc.gpsimd.dma_start(s["mk"][:, c, :], maskT[p, :, c, :])

            def tile_work(p, c, prev):
                """QK + exp + mask + PV for (p, c); interleave pair prev's
                attention normalize+store for the same tile index."""
                s = st[p]
                if c == 0:
                    s["pm"] = pmp.tile([128, KT, LQ], BF16, tag="pm",
                                       name=f"pm{p}")
                    s["pv"] = ps_pv.tile([128, LQ], F32, tag="pv",
                                         name=f"pv{p}")
                pm, pv = s["pm"], s["pv"]
                ps = ps_sc.tile([128, 512], F32, tag="ps", name=f"ps{p}_{c}a")
                ps2 = ps_sc.tile([128, 512], F32, tag="ps", name=f"ps{p}_{c}b")
                nc.tensor.matmul(ps[:], s["kt"][:, c * 128:(c + 1) * 128],
                                 s["qt"][:, 0:512], start=True, stop=True)
                nc.tensor.matmul(ps2[:], s["kt"][:, c * 128:(c + 1) * 128],
                                 s["qt"][:, 512:1024], start=True, stop=True)
                nc.scalar.activation(pm[:, c, 0:512], ps[:], AF.Exp, scale=SCALE)
                nc.scalar.activation(pm[:, c, 512:1024], ps2[:], AF.Exp,
                                     scale=SCALE)
                nc.vector.tensor_mul(pm[:, c, :], pm[:, c, :], s["mk"][:, c, :])
                for h in range(2):
                    sl = slice(h * 512, (h + 1) * 512)
                    nc.tensor.matmul(pv[0:D + 1, sl], s["vt"][:, c, :],
                                     pm[:, c, sl], start=(c == 0),
                                     stop=(c == KT - 1))
                if prev is not None:
                    sp = st[prev]
                    att = attp.tile([128, LQ], BF16, tag="att")
                    nc.vector.tensor_mul(att[:], sp["pm"][:, c, :], sp["R_sb"][:])
                    eng = nc.sync if (c % 2 == 0) else nc.scalar
                    eng.dma_start(attT[prev, c * 128:(c + 1) * 128, :], att[:])

            def evac(p):
                # evacuate pv rows (frees the single pv PSUM slot quickly)
                s = st[p]
                s["s_row"] = smallp.tile([128, 512], F32, tag="s_row",
                                         name=f"s_row{p}")
                nc.vector.memset(s["s_row"][:], 1.0)
                for j in range(2):
                    nc.scalar.activation(
                        s["s_row"][64 * j:64 * j + 1, :],
                        s["pv"][D:D + 1, 512 * j:512 * (j + 1)], AF.Copy)
                s["outsb"] = smallp.tile([D, LQ], BF16, tag="outsb",
                                         name=f"outsb{p}")
                nc.scalar.activation(s["outsb"][:], s["pv"][0:D, :], AF.Copy)
                del s["pv"]

            def recip_chain(p):
                s = st[p]
                rec = smallp.tile([128, 512], F32, tag="rec")
                nc.vector.reciprocal(rec[:], s["s_row"][:])
                rec2b = smallp.tile([128, 512], BF16, tag="rec2b")
                nc.vector.tensor_mul(rec2b[:], rec[:], s["qmt"][:])

                # R[k, 512j+i] = rec2b[64j, i] for all k: rank-1 matmuls with
                # both operands on partition 64j (ones row x recip chunk)
                R = ps_rr.tile([128, LQ], F32, tag="rr")
                for j in range(2):
                    nc.tensor.matmul(R[:, 512 * j:512 * (j + 1)],
                                     ones128_b[64 * j:64 * j + 1, :],
                                     rec2b[64 * j:64 * j + 1, :],
                                     start=True, stop=True)
                s["R_sb"] = attp.tile([128, LQ], BF16, tag="rsb",
                                      name=f"rsb{p}")
                nc.scalar.activation(s["R_sb"][:], R[:], AF.Copy)

                outn = smallp.tile([D, LQ], F32, tag="outn")
                nc.vector.tensor_mul(outn[:], s["outsb"][:], s["R_sb"][0:D, :])
                nc.scalar.dma_start(outT[p], outn[:])

            # software-pipelined emission
            load(0)
            for c in range(KT):
                tile_work(0, c, None)
            evac(0)
            for p in range(1, PAIRS):
                load(p)
                recip_chain(p - 1)
                for c in range(KT):
                    tile_work(p, c, p - 1)
                evac(p)
                st[p - 1] = {}
            recip_chain(PAIRS - 1)
            sp = st[PAIRS - 1]
            for c in range(KT):
                att = attp.tile([128, LQ], BF16, tag="att")
                nc.vector.tensor_mul(att[:], sp["pm"][:, c, :], sp["R_sb"][:])
                eng = nc.sync if (c % 2 == 0) else nc.scalar
                eng.dma_start(attT[PAIRS - 1, c * 128:(c + 1) * 128, :], att[:])

    nc.finalize()
    return nc


def _get_nc():
    if "nc" not in _compiled:
        _compiled["nc"] = _build_bass()
    return _compiled["nc"]


def make_in_maps(query, key, value, mask, query_mask):
    query = np.asarray(query, dtype=np.float32)
    key = np.asarray(key, dtype=np.float32)
    value = np.asarray(value, dtype=np.float32)
    mask = np.asarray(mask, dtype=np.int32)
    query_mask = np.asarray(query_mask, dtype=np.float32)

    q_f = query.reshape(B * H, LQ, D)
    k_f = key.reshape(B * H, LK, D)
    v_f = value.reshape(B * H, LK, D)
    m_f = mask.reshape(B * H, LQ, LK)
    qm_f = query_mask.reshape(B * H, LQ)

    in_maps = []
    for i in range(N_CORES):
        sl = slice(i * PAIRS, (i + 1) * PAIRS)
        qT_i = np.ascontiguousarray(q_f[sl].transpose(0, 2, 1))          # [P, D, LQ]
        kT_i = np.ascontiguousarray(k_f[sl].transpose(0, 2, 1))          # [P, D, LK]
        v_c = v_f[sl].reshape(PAIRS, KT, 128, D).transpose(0, 2, 1, 3)   # [P,128,KT,D]
        v_i = np.concatenate(
            [v_c, np.ones((PAIRS, 128, KT, 1), dtype=np.float32)], axis=-1)
        v_i = np.ascontiguousarray(v_i)                                  # [P,128,KT,D+1]
        mT = m_f[sl].transpose(0, 2, 1)                                  # [P, k, q]
        mT_i = np.ascontiguousarray(
            mT.reshape(PAIRS, KT, 128, LQ).transpose(0, 2, 1, 3))        # [P,128,KT,q]
        # query_mask halves on partitions 0 and 64 of a [128,512] tile
        qm_i = np.zeros((PAIRS, 128, 512), dtype=np.float32)
        qm_i[:, 0, :] = qm_f[sl][:, 0:512]
        qm_i[:, 64, :] = qm_f[sl][:, 512:1024]
        in_maps.append({"qT": qT_i, "kT": kT_i, "v": v_i,
                        "maskT": mT_i, "qmc": qm_i})
    return in_maps


def _axon_reset():
    try:
        import ctypes
        lib = ctypes.CDLL('/opt/axon/libaxon_pjrt.so')
        lib.axon_reset.restype = ctypes.c_int64
        lib.axon_reset()
    except Exception:
        pass


def kernel(query, key, value, mask, query_mask):
    from concourse.bass_utils import run_bass_kernel_spmd

    in_maps = make_in_maps(query, key, value, mask, query_mask)
    nc = _get_nc()
    try:
        res = run_bass_kernel_spmd(nc, in_maps, core_ids=list(range(N_CORES)))
    except Exception:
        # device pool may be wedged from a prior run — reset and retry once
        _axon_reset()
        res = run_bass_kernel_spmd(nc, in_maps, core_ids=list(range(N_CORES)))

    att_T = np.stack([np.asarray(res.results[i]["attT"], dtype=np.float32)
                      for i in range(N_CORES)])                          # [8,P,k,q]
    out_T = np.stack([np.asarray(res.results[i]["outT"], dtype=np.float32)
                      for i in range(N_CORES)])                          # [8,P,D,LQ]

    attention = att_T.reshape(B, H, LK, LQ).swapaxes(2, 3)               # [B,H,q,k]
    output = out_T.reshape(B, H, D, LQ).swapaxes(2, 3)                   # [B,H,q,D]
    return np.ascontiguousarray(output), np.ascontiguousarray(attention)


# revision 24
# speedup vs baseline: 1.0818x; 1.0818x over previous
"""Sparse attention (B=8,H=8,LQ=LK=1024,D=64) on 8 TRN2 NeuronCores.

Strategy: shard the 64 (b,h) pairs across 8 cores (8 pairs/core) — fully
independent, no collectives. On each core, compute in the TRANSPOSED
domain: scores^T [k, q] tiles so that the P@V contraction (over k) needs
no on-chip transposes; Q^T / K^T / mask^T are produced host-side during
sharding (pure layout), outputs are un-transposed host-side after gather.

Per (b,h) pair on device:
  S^T[k,q]   = K^T_tile.T @ Q^T     (PE, f32r)
  P          = exp(S^T / 8)         (ScalarE, PSUM->SBUF, bf16 out)
  Pm         = P * mask^T           (VectorE, int32 operand, in-place)
  out'^T     = [V | 1].T @ Pm       (PE accumulate; row 64 = rowsums)
  recip      = 1/rowsums, redistributed [1,1024]->[128,8] via tiny MMs
  R          = broadcast(recip * qmask) across partitions via diag-matmul
  att^T      = Pm * R_bf16          (VectorE 2x mode, bf16 out) -> DMA
  out^T      = out'^T[0:64] * R[0:64] -> DMA

Emission is software-pipelined and interleaved at k-tile granularity:
pair p-1's normalize work is emitted tile-by-tile between pair p's
tiles so every engine's in-order stream always has ready work.
"""

import numpy as np

B, H, LQ, LK, D = 8, 8, 1024, 1024, 64
N_CORES = 8
PAIRS = (B * H) // N_CORES          # 8 pairs per core
KT = LK // 128                      # 8 k-tiles of 128
SCALE = 1.0 / 8.0                   # 1/sqrt(64)

_compiled = {}


def _build_bass():
    import concourse.tile as tile
    import concourse.bacc as bacc
    import concourse.mybir as mybir
    from concourse.masks import make_identity

    F32 = mybir.dt.float32
    F32R = mybir.dt.float32r
    BF16 = mybir.dt.bfloat16
    I32 = mybir.dt.int32
    AF = mybir.ActivationFunctionType

    nc = bacc.Bacc("TRN2", target_bir_lowering=False, debug=False,
                   num_devices=N_CORES)

    qT = nc.dram_tensor("qT", [PAIRS, D, LQ], F32R, kind="ExternalInput")
    kT = nc.dram_tensor("kT", [PAIRS, D, LK], F32R, kind="ExternalInput")
    v = nc.dram_tensor("v", [PAIRS, 128, KT, D + 1], F32, kind="ExternalInput")
    maskT = nc.dram_tensor("maskT", [PAIRS, 128, KT, LQ], I32, kind="ExternalInput")
    qmc = nc.dram_tensor("qmc", [PAIRS, 128, 512], F32, kind="ExternalInput")
    attT = nc.dram_tensor("attT", [PAIRS, LK, LQ], BF16, kind="ExternalOutput")
    outT = nc.dram_tensor("outT", [PAIRS, D, LQ], F32, kind="ExternalOutput")

    with tile.TileContext(nc) as tc:
        with (
            tc.tile_pool(name="constp", bufs=1) as constp,
            tc.tile_pool(name="qkp", bufs=2) as qkp,
            tc.tile_pool(name="vp", bufs=2) as vp,
            tc.tile_pool(name="maskp", bufs=2) as maskp,
            tc.tile_pool(name="pmp", bufs=2) as pmp,
            tc.tile_pool(name="attp", bufs=3) as attp,
            tc.tile_pool(name="smallp", bufs=2) as smallp,
            tc.tile_pool(name="ps_sc", bufs=4, space="PSUM") as ps_sc,
            tc.tile_pool(name="ps_rr", bufs=1, space="PSUM") as ps_rr,
            tc.tile_pool(name="ps_pv", bufs=1, space="PSUM") as ps_pv,
        ):
            # constants
            ones_f32 = constp.tile([128, 128], F32)
            nc.vector.memset(ones_f32[:], 1.0)
            ones128_b = constp.tile([128, 128], BF16)
            nc.vector.tensor_copy(ones128_b[:], ones_f32[:])

            # HAM warmup: ~5us of back-to-back matmuls so the PE clock
            # ungates to 2.4GHz before real work begins.
            wu_rhs = constp.tile([128, 512], BF16)
            nc.vector.memset(wu_rhs[:], 0.5)
            wu_ps = ps_sc.tile([128, 512], F32, tag="ps", name="wu_ps")
            for _ in range(12):
                nc.tensor.matmul(wu_ps[:], ones128_b[:], wu_rhs[:],
                                 start=True, stop=True)

            st = [dict() for _ in range(PAIRS)]   # per-pair live tiles

            def load(p):
                s = st[p]
                s["qt"] = qkp.tile([D, LQ], F32R, tag="qt", name=f"qt{p}")
                s["kt"] = qkp.tile([D, LK], F32R, tag="kt", name=f"kt{p}")
                s["vt"] = vp.tile([128, KT, D + 1], BF16, tag="vt", name=f"vt{p}")
                # bf16 tile filled by SWDGE cast-DMA from the int32 mask —
                # HBM reads unchanged, halves SBUF + enables DVE 2x mode
                s["mk"] = maskp.tile([128, KT, LQ], BF16, tag="mk", name=f"mk{p}")
                s["qmt"] = smallp.tile([128, 512], F32, tag="qmt", name=f"qmt{p}")
                nc.sync.dma_start(s["qt"][:], qT[p])
                nc.sync.dma_start(s["kt"][:], kT[p])
                nc.sync.dma_start(s["qmt"][:], qmc[p])
                nc.gpsimd.dma_start(s["vt"][:], v[p])   # SWDGE cast f32->bf16
                # mask per k-tile so the first tile's compute starts early
                for c in range(KT):
                    nc.gpsimd.dma_start(s["mk"][:, c, :], maskT[p, :, c, :])

            def tile_work(p, c, prev):
                """QK + exp + mask + PV for (p, c); interleave pair prev's
                attention normalize+store for the same tile index."""
                s = st[p]
                if c == 0:
                    s["pm"] = pmp.tile([128, KT, LQ], BF16, tag="pm",
                                       name=f"pm{p}")
                    s["pv"] = ps_pv.tile([128, LQ], F32, tag="pv",
                                         name=f"pv{p}")
                pm, pv = s["pm"], s["pv"]
                ps = ps_sc.tile([128, 512], F32, tag="ps", name=f"ps{p}_{c}a")
                ps2 = ps_sc.tile([128, 512], F32, tag="ps", name=f"ps{p}_{c}b")
                nc.tensor.matmul(ps[:], s["kt"][:, c * 128:(c + 1) * 128],
                                 s["qt"][:, 0:512], start=True, stop=True)
                nc.tensor.matmul(ps2[:], s["kt"][:, c * 128:(c + 1) * 128],
                                 s["qt"][:, 512:1024], start=True, stop=True)
                nc.scalar.activation(pm[:, c, 0:512], ps[:], AF.Exp, scale=SCALE)
                nc.scalar.activation(pm[:, c, 512:1024], ps2[:], AF.Exp,
                                     scale=SCALE)
                nc.vector.tensor_mul(pm[:, c, :], pm[:, c, :], s["mk"][:, c, :])
                for h in range(2):
                    sl = slice(h * 512, (h + 1) * 512)
                    nc.tensor.matmul(pv[0:D + 1, sl], s["vt"][:, c, :],
                                     pm[:, c, sl], start=(c == 0),
                                     stop=(c == KT - 1))
                if prev is not None and c >= 2:
                    emit_att(prev, c - 2)

            def evac(p):
                # evacuate pv rows (frees the single pv PSUM slot quickly)
                s = st[p]
                s["s_row"] = smallp.tile([128, 512], F32, tag="s_row",
                                         name=f"s_row{p}")
                nc.vector.memset(s["s_row"][:], 1.0)
                for j in range(2):
                    nc.scalar.activation(
                        s["s_row"][64 * j:64 * j + 1, :],
                        s["pv"][D:D + 1, 512 * j:512 * (j + 1)], AF.Copy)
                s["outsb"] = smallp.tile([D, LQ], BF16, tag="outsb",
                                         name=f"outsb{p}")
                nc.scalar.activation(s["outsb"][:], s["pv"][0:D, :], AF.Copy)
                del s["pv"]

            def recip_chain(p):
                s = st[p]
                rec = smallp.tile([128, 512], F32, tag="rec")
                nc.vector.reciprocal(rec[:], s["s_row"][:])
                rec2b = smallp.tile([128, 512], BF16, tag="rec2b")
                nc.vector.tensor_mul(rec2b[:], rec[:], s["qmt"][:])

                # R[k, 512j+i] = rec2b[64j, i] for all k: rank-1 matmuls with
                # both operands on partition 64j (ones row x recip chunk)
                R = ps_rr.tile([128, LQ], F32, tag="rr")
                for j in range(2):
                    nc.tensor.matmul(R[:, 512 * j:512 * (j + 1)],
                                     ones128_b[64 * j:64 * j + 1, :],
                                     rec2b[64 * j:64 * j + 1, :],
                                     start=True, stop=True)
                s["R_sb"] = attp.tile([128, LQ], BF16, tag="rsb",
                                      name=f"rsb{p}")
                nc.scalar.activation(s["R_sb"][:], R[:], AF.Copy)

                outn = smallp.tile([D, LQ], F32, tag="outn")
                nc.vector.tensor_mul(outn[:], s["outsb"][:], s["R_sb"][0:D, :])
                nc.scalar.dma_start(outT[p], outn[:])

            def emit_att(p, c):
                sp = st[p]
                att = attp.tile([128, LQ], BF16, tag="att")
                nc.vector.tensor_mul(att[:], sp["pm"][:, c, :], sp["R_sb"][:])
                eng = nc.sync if (c % 2 == 0) else nc.scalar
                eng.dma_start(attT[p, c * 128:(c + 1) * 128, :], att[:])

            # software-pipelined emission: pair p-1's recip chain is emitted
            # after two of pair p's tiles are queued, and its attention
            # tiles trail by two tile slots, so the DVE stream never stalls.
            load(0)
            for c in range(KT):
                tile_work(0, c, None)
            evac(0)
            for p in range(1, PAIRS):
                load(p)
                tile_work(p, 0, p - 1)
                tile_work(p, 1, p - 1)
                recip_chain(p - 1)
                for c in range(2, KT):
                    tile_work(p, c, p - 1)
                emit_att(p - 1, KT - 2)
                emit_att(p - 1, KT - 1)
                evac(p)
                st[p - 1] = {}
            recip_chain(PAIRS - 1)
            for c in range(KT):
                emit_att(PAIRS - 1, c)

    nc.finalize()
    return nc


def _get_nc():
    if "nc" not in _compiled:
        _compiled["nc"] = _build_bass()
    return _compiled["nc"]


def make_in_maps(query, key, value, mask, query_mask):
    query = np.asarray(query, dtype=np.float32)
    key = np.asarray(key, dtype=np.float32)
    value = np.asarray(value, dtype=np.float32)
    mask = np.asarray(mask, dtype=np.int32)
    query_mask = np.asarray(query_mask, dtype=np.float32)

    q_f = query.reshape(B * H, LQ, D)
    k_f = key.reshape(B * H, LK, D)
    v_f = value.reshape(B * H, LK, D)
    m_f = mask.reshape(B * H, LQ, LK)
    qm_f = query_mask.reshape(B * H, LQ)

    in_maps = []
    for i in range(N_CORES):
        sl = slice(i * PAIRS, (i + 1) * PAIRS)
        qT_i = np.ascontiguousarray(q_f[sl].transpose(0, 2, 1))          # [P, D, LQ]
        kT_i = np.ascontiguousarray(k_f[sl].transpose(0, 2, 1))          # [P, D, LK]
        v_c = v_f[sl].reshape(PAIRS, KT, 128, D).transpose(0, 2, 1, 3)   # [P,128,KT,D]
        v_i = np.concatenate(
            [v_c, np.ones((PAIRS, 128, KT, 1), dtype=np.float32)], axis=-1)
        v_i = np.ascontiguousarray(v_i)                                  # [P,128,KT,D+1]
        mT = m_f[sl].transpose(0, 2, 1)                                  # [P, k, q]
        mT_i = np.ascontiguousarray(
            mT.reshape(PAIRS, KT, 128, LQ).transpose(0, 2, 1, 3))        # [P,128,KT,q]
        # query_mask halves on partitions 0 and 64 of a [128,512] tile
        qm_i = np.zeros((PAIRS, 128, 512), dtype=np.float32)
        qm_i[:, 0, :] = qm_f[sl][:, 0:512]
        qm_i[:, 64, :] = qm_f[sl][:, 512:1024]
        in_maps.append({"qT": qT_i, "kT": kT_i, "v": v_i,
                        "maskT": mT_i, "qmc": qm_i})
    return in_maps


def _axon_reset():
    try:
        import ctypes
        lib = ctypes.CDLL('/opt/axon/libaxon_pjrt.so')
        lib.axon_reset.restype = ctypes.c_int64
        lib.axon_reset()
    except Exception:
        pass


def kernel(query, key, value, mask, query_mask):
    from concourse.bass_utils import run_bass_kernel_spmd

    in_maps = make_in_maps(query, key, value, mask, query_mask)
    nc = _get_nc()
    try:
        res = run_bass_kernel_spmd(nc, in_maps, core_ids=list(range(N_CORES)))
    except Exception:
        # device pool may be wedged from a prior run — reset and retry once
        _axon_reset()
        res = run_bass_kernel_spmd(nc, in_maps, core_ids=list(range(N_CORES)))

    att_T = np.stack([np.asarray(res.results[i]["attT"], dtype=np.float32)
                      for i in range(N_CORES)])                          # [8,P,k,q]
    out_T = np.stack([np.asarray(res.results[i]["outT"], dtype=np.float32)
                      for i in range(N_CORES)])                          # [8,P,D,LQ]

    attention = att_T.reshape(B, H, LK, LQ).swapaxes(2, 3)               # [B,H,q,k]
    output = out_T.reshape(B, H, D, LQ).swapaxes(2, 3)                   # [B,H,q,D]
    return np.ascontiguousarray(output), np.ascontiguousarray(attention)


# revision 25
# speedup vs baseline: 1.1972x; 1.1067x over previous
"""Sparse attention (B=8,H=8,LQ=LK=1024,D=64) on 8 TRN2 NeuronCores.

Strategy: shard the 64 (b,h) pairs across 8 cores (8 pairs/core) — fully
independent, no collectives. On each core, compute in the TRANSPOSED
domain: scores^T [k, q] tiles so that the P@V contraction (over k) needs
no on-chip transposes; Q^T / K^T / mask^T are produced host-side during
sharding (pure layout), outputs are un-transposed host-side after gather.

Per (b,h) pair on device:
  S^T[k,q]   = K^T_tile.T @ Q^T     (PE, f32r)
  P          = exp(S^T / 8)         (ScalarE, PSUM->SBUF, bf16 out)
  Pm         = P * mask^T           (VectorE, int32 operand, in-place)
  out'^T     = [V | 1].T @ Pm       (PE accumulate; row 64 = rowsums)
  recip      = 1/rowsums, redistributed [1,1024]->[128,8] via tiny MMs
  R          = broadcast(recip * qmask) across partitions via diag-matmul
  att^T      = Pm * R_bf16          (VectorE 2x mode, bf16 out) -> DMA
  out^T      = out'^T[0:64] * R[0:64] -> DMA

Emission is software-pipelined and interleaved at k-tile granularity:
pair p-1's normalize work is emitted tile-by-tile between pair p's
tiles so every engine's in-order stream always has ready work.
"""

import numpy as np

B, H, LQ, LK, D = 8, 8, 1024, 1024, 64
N_CORES = 8
PAIRS = (B * H) // N_CORES          # 8 pairs per core
KT = LK // 128                      # 8 k-tiles of 128
SCALE = 1.0 / 8.0                   # 1/sqrt(64)

_compiled = {}


def _build_bass():
    import concourse.tile as tile
    import concourse.bacc as bacc
    import concourse.mybir as mybir
    from concourse.masks import make_identity

    F32 = mybir.dt.float32
    F32R = mybir.dt.float32r
    BF16 = mybir.dt.bfloat16
    I32 = mybir.dt.int32
    AF = mybir.ActivationFunctionType

    nc = bacc.Bacc("TRN2", target_bir_lowering=False, debug=False,
                   num_devices=N_CORES)

    qT = nc.dram_tensor("qT", [PAIRS, D, LQ], F32R, kind="ExternalInput")
    kT = nc.dram_tensor("kT", [PAIRS, D, LK], F32R, kind="ExternalInput")
    v = nc.dram_tensor("v", [PAIRS, 128, KT, D + 1], F32, kind="ExternalInput")
    maskT = nc.dram_tensor("maskT", [PAIRS, 128, KT, LQ], I32, kind="ExternalInput")
    qmc = nc.dram_tensor("qmc", [PAIRS, 2, 512], F32, kind="ExternalInput")
    attT = nc.dram_tensor("attT", [PAIRS, LK, LQ], BF16, kind="ExternalOutput")
    outT = nc.dram_tensor("outT", [PAIRS, D, LQ], F32, kind="ExternalOutput")

    with tile.TileContext(nc) as tc:
        with (
            tc.tile_pool(name="constp", bufs=1) as constp,
            tc.tile_pool(name="qkp", bufs=2) as qkp,
            tc.tile_pool(name="vp", bufs=2) as vp,
            tc.tile_pool(name="maskp", bufs=2) as maskp,
            tc.tile_pool(name="pmp", bufs=2) as pmp,
            tc.tile_pool(name="attp", bufs=3) as attp,
            tc.tile_pool(name="smallp", bufs=2) as smallp,
            tc.tile_pool(name="ps_sc", bufs=4, space="PSUM") as ps_sc,
            tc.tile_pool(name="ps_rr", bufs=1, space="PSUM") as ps_rr,
            tc.tile_pool(name="ps_pv", bufs=1, space="PSUM") as ps_pv,
        ):
            # constants
            ones_f32 = constp.tile([128, 128], F32)
            nc.vector.memset(ones_f32[:], 1.0)
            ones128_b = constp.tile([128, 128], BF16)
            nc.vector.tensor_copy(ones128_b[:], ones_f32[:])

            # HAM warmup: ~5us of back-to-back matmuls so the PE clock
            # ungates to 2.4GHz before real work begins.
            wu_rhs = constp.tile([128, 512], BF16)
            nc.vector.memset(wu_rhs[:], 0.5)
            wu_ps = ps_sc.tile([128, 512], F32, tag="ps", name="wu_ps")
            for _ in range(12):
                nc.tensor.matmul(wu_ps[:], ones128_b[:], wu_rhs[:],
                                 start=True, stop=True)

            st = [dict() for _ in range(PAIRS)]   # per-pair live tiles

            def load(p):
                s = st[p]
                s["qt"] = qkp.tile([D, LQ], F32R, tag="qt", name=f"qt{p}")
                s["kt"] = qkp.tile([D, LK], F32R, tag="kt", name=f"kt{p}")
                s["vt"] = vp.tile([128, KT, D + 1], BF16, tag="vt", name=f"vt{p}")
                # bf16 tile filled by SWDGE cast-DMA from the int32 mask —
                # HBM reads unchanged, halves SBUF + enables DVE 2x mode
                s["mk"] = maskp.tile([128, KT, LQ], BF16, tag="mk", name=f"mk{p}")
                s["qmt"] = smallp.tile([128, 512], F32, tag="qmt", name=f"qmt{p}")
                nc.sync.dma_start(s["qt"][:], qT[p])
                nc.sync.dma_start(s["kt"][:], kT[p])
                nc.sync.dma_start(s["qmt"][0:1, :], qmc[p, 0:1, :])
                nc.sync.dma_start(s["qmt"][64:65, :], qmc[p, 1:2, :])
                nc.gpsimd.dma_start(s["vt"][:], v[p])   # SWDGE cast f32->bf16
                # mask in two 2MB batches: efficient and still lets the
                # first half-pair's compute start early
                for g in range(2):
                    nc.gpsimd.dma_start(s["mk"][:, 4 * g:4 * g + 4, :],
                                        maskT[p, :, 4 * g:4 * g + 4, :])

            def tile_work(p, c, prev):
                """QK + exp + mask + PV for (p, c); interleave pair prev's
                attention normalize+store for the same tile index."""
                s = st[p]
                if c == 0:
                    s["pm"] = pmp.tile([128, KT, LQ], BF16, tag="pm",
                                       name=f"pm{p}")
                    s["pv"] = ps_pv.tile([128, LQ], F32, tag="pv",
                                         name=f"pv{p}")
                pm, pv = s["pm"], s["pv"]
                ps = ps_sc.tile([128, 512], F32, tag="ps", name=f"ps{p}_{c}a")
                ps2 = ps_sc.tile([128, 512], F32, tag="ps", name=f"ps{p}_{c}b")
                nc.tensor.matmul(ps[:], s["kt"][:, c * 128:(c + 1) * 128],
                                 s["qt"][:, 0:512], start=True, stop=True)
                nc.tensor.matmul(ps2[:], s["kt"][:, c * 128:(c + 1) * 128],
                                 s["qt"][:, 512:1024], start=True, stop=True)
                nc.scalar.activation(pm[:, c, 0:512], ps[:], AF.Exp, scale=SCALE)
                nc.scalar.activation(pm[:, c, 512:1024], ps2[:], AF.Exp,
                                     scale=SCALE)
                nc.vector.tensor_mul(pm[:, c, :], pm[:, c, :], s["mk"][:, c, :])
                for h in range(2):
                    sl = slice(h * 512, (h + 1) * 512)
                    nc.tensor.matmul(pv[0:D + 1, sl], s["vt"][:, c, :],
                                     pm[:, c, sl], start=(c == 0),
                                     stop=(c == KT - 1))
                if prev is not None and c >= 2:
                    emit_att(prev, c - 2)

            def evac(p):
                # evacuate pv rows (frees the single pv PSUM slot quickly)
                s = st[p]
                s["s_row"] = smallp.tile([128, 512], F32, tag="s_row",
                                         name=f"s_row{p}")
                nc.vector.memset(s["s_row"][:], 1.0)
                for j in range(2):
                    nc.scalar.activation(
                        s["s_row"][64 * j:64 * j + 1, :],
                        s["pv"][D:D + 1, 512 * j:512 * (j + 1)], AF.Copy)
                s["outsb"] = smallp.tile([D, LQ], BF16, tag="outsb",
                                         name=f"outsb{p}")
                nc.scalar.activation(s["outsb"][:], s["pv"][0:D, :], AF.Copy)
                del s["pv"]

            def recip_chain(p):
                s = st[p]
                rec = smallp.tile([128, 512], F32, tag="rec")
                nc.vector.reciprocal(rec[:], s["s_row"][:])
                rec2b = smallp.tile([128, 512], BF16, tag="rec2b")
                nc.vector.tensor_mul(rec2b[:], rec[:], s["qmt"][:])

                # R[k, 512j+i] = rec2b[64j, i] for all k: rank-1 matmuls with
                # both operands on partition 64j (ones row x recip chunk)
                R = ps_rr.tile([128, LQ], F32, tag="rr")
                for j in range(2):
                    nc.tensor.matmul(R[:, 512 * j:512 * (j + 1)],
                                     ones128_b[64 * j:64 * j + 1, :],
                                     rec2b[64 * j:64 * j + 1, :],
                                     start=True, stop=True)
                s["R_sb"] = attp.tile([128, LQ], BF16, tag="rsb",
                                      name=f"rsb{p}")
                nc.scalar.activation(s["R_sb"][:], R[:], AF.Copy)

                outn = smallp.tile([D, LQ], F32, tag="outn")
                nc.vector.tensor_mul(outn[:], s["outsb"][:], s["R_sb"][0:D, :])
                nc.scalar.dma_start(outT[p], outn[:])

            def emit_att(p, c):
                sp = st[p]
                att = attp.tile([128, LQ], BF16, tag="att")
                nc.vector.tensor_mul(att[:], sp["pm"][:, c, :], sp["R_sb"][:])
                eng = nc.sync if (c % 2 == 0) else nc.scalar
                eng.dma_start(attT[p, c * 128:(c + 1) * 128, :], att[:])

            # software-pipelined emission: pair p-1's recip chain is emitted
            # after two of pair p's tiles are queued, and its attention
            # tiles trail by two tile slots, so the DVE stream never stalls.
            load(0)
            for c in range(KT):
                tile_work(0, c, None)
            evac(0)
            for p in range(1, PAIRS):
                load(p)
                tile_work(p, 0, p - 1)
                tile_work(p, 1, p - 1)
                recip_chain(p - 1)
                for c in range(2, KT):
                    tile_work(p, c, p - 1)
                emit_att(p - 1, KT - 2)
                emit_att(p - 1, KT - 1)
                evac(p)
                st[p - 1] = {}
            recip_chain(PAIRS - 1)
            for c in range(KT):
                emit_att(PAIRS - 1, c)

    nc.finalize()
    return nc


def _get_nc():
    if "nc" not in _compiled:
        _compiled["nc"] = _build_bass()
    return _compiled["nc"]


def make_in_maps(query, key, value, mask, query_mask):
    query = np.asarray(query, dtype=np.float32)
    key = np.asarray(key, dtype=np.float32)
    value = np.asarray(value, dtype=np.float32)
    mask = np.asarray(mask, dtype=np.int32)
    query_mask = np.asarray(query_mask, dtype=np.float32)

    q_f = query.reshape(B * H, LQ, D)
    k_f = key.reshape(B * H, LK, D)
    v_f = value.reshape(B * H, LK, D)
    m_f = mask.reshape(B * H, LQ, LK)
    qm_f = query_mask.reshape(B * H, LQ)

    in_maps = []
    for i in range(N_CORES):
        sl = slice(i * PAIRS, (i + 1) * PAIRS)
        qT_i = np.ascontiguousarray(q_f[sl].transpose(0, 2, 1))          # [P, D, LQ]
        kT_i = np.ascontiguousarray(k_f[sl].transpose(0, 2, 1))          # [P, D, LK]
        v_c = v_f[sl].reshape(PAIRS, KT, 128, D).transpose(0, 2, 1, 3)   # [P,128,KT,D]
        v_i = np.concatenate(
            [v_c, np.ones((PAIRS, 128, KT, 1), dtype=np.float32)], axis=-1)
        v_i = np.ascontiguousarray(v_i)                                  # [P,128,KT,D+1]
        mT = m_f[sl].transpose(0, 2, 1)                                  # [P, k, q]
        mT_i = np.ascontiguousarray(
            mT.reshape(PAIRS, KT, 128, LQ).transpose(0, 2, 1, 3))        # [P,128,KT,q]
        # query_mask halves -> device rows 0 and 64 of the qmt tile
        qm_i = np.ascontiguousarray(
            qm_f[sl].reshape(PAIRS, 2, 512))
        in_maps.append({"qT": qT_i, "kT": kT_i, "v": v_i,
                        "maskT": mT_i, "qmc": qm_i})
    return in_maps


def _axon_reset():
    try:
        import ctypes
        lib = ctypes.CDLL('/opt/axon/libaxon_pjrt.so')
        lib.axon_reset.restype = ctypes.c_int64
        lib.axon_reset()
    except Exception:
        pass


def kernel(query, key, value, mask, query_mask):
    from concourse.bass_utils import run_bass_kernel_spmd

    in_maps = make_in_maps(query, key, value, mask, query_mask)
    nc = _get_nc()
    try:
        res = run_bass_kernel_spmd(nc, in_maps, core_ids=list(range(N_CORES)))
    except Exception:
        # device pool may be wedged from a prior run — reset and retry once
        _axon_reset()
        res = run_bass_kernel_spmd(nc, in_maps, core_ids=list(range(N_CORES)))

    att_T = np.stack([np.asarray(res.results[i]["attT"], dtype=np.float32)
                      for i in range(N_CORES)])                          # [8,P,k,q]
    out_T = np.stack([np.asarray(res.results[i]["outT"], dtype=np.float32)
                      for i in range(N_CORES)])                          # [8,P,D,LQ]

    attention = att_T.reshape(B, H, LK, LQ).swapaxes(2, 3)               # [B,H,q,k]
    output = out_T.reshape(B, H, D, LQ).swapaxes(2, 3)                   # [B,H,q,D]
    return np.ascontiguousarray(output), np.ascontiguousarray(attention)
